# revision 72
# baseline (speedup 1.0000x reference)
"""Trainium2 Bass kernel for nn_Cascade_CNN_RNN (cascade CNN -> MGU RNN).

Data-parallel over batch across 8 NeuronCores. Per core (shard B=256):
  - x quantized on HOST (round-half-even to k/128, exact in bf16) and
    uploaded as bf16 [110, T*BS]
  - conv1 as banded spatial-operator matmuls -> a1 in 30 slots of
    4 rows x 2 cols [p = ci*8 + di*2 + dc], slot s = cp*5 + rw; the
    4-row window equals the conv2 tile RF height so each standard conv2
    tile contracts exactly 2 slots with 2 shared lhsT
  - conv2: 25 standard 2x2 tiles (2 K-slots) + 3 j=10 tiles (5 MMs
    total, 2 more shared lhsT) -> F [128, 28, Sc]
  - fc3 28 K-chunks + gi (single M=128 accumulation pair), all
    column-interleaved with the conv stream so every engine pipelines;
    epilogues split ACT/DVE by static patterns
  - gi and MGU windows deferred into the NEXT chunk's PE stream; gh is
    one M=128 matmul read by DVE at PSUM partition offsets 0/64; hq
    magic-round quantize on the ACT engine (fp32-exact scale+bias)
  - chunks of [1,2,2,2,2,1] windows (small first chunk hides in the
    DMA/preamble ramp); tail = window 9 chain + fc5
All matmul-facing tensors are bf16 (full-speed PE + FWL weight loads);
PSUM accumulation is fp32, MGU recurrence state fp32 (whh as f32r).
"""

import numpy as np
import ml_dtypes

import concourse.bass as bass
import concourse.mybir as mybir
import concourse.tile as tile
from concourse import bacc
from concourse.bass_utils import run_bass_kernel_spmd

F32 = mybir.dt.float32
F32R = mybir.dt.float32r
BF16 = mybir.dt.bfloat16
BF16NP = ml_dtypes.bfloat16
MAGIC = 12582912.0  # 1.5 * 2**23: fp32 round-to-nearest-even integer trick
INV_SCALE = 0.0078125  # 1/128

T, HH, WW = 10, 10, 11  # windows, height, width
SP = HH * WW  # 110 input spatial positions
CH1, CH2 = 16, 32
NCLS = 7
HID = 64

NSLOT = 30   # conv1 slots: s = cp*5 + rw (col-pair major, 4x2 positions)
NK = 28      # fc3 K-chunks / conv2 output tiles

# epilogue engine split patterns (True -> ACT/scalar, False -> DVE),
# keyed by windows-per-chunk (small chunks are elementwise-bound, so the
# DVE keeps a lighter share there)
C1_PAT = {2: [True, True, False], 1: [True, True, True, False]}
C2_PAT = {2: [True, False], 1: [True, False]}


# ---------------------------------------------------------------- host packing
def _pack_weights(conv1_w, conv2_w, fc3_w, w_ih, w_hh, fc5_w):
    # conv1 operator lhsT: [k=(yy*11+xx), slot=(cp*5+rw), p=(ci*8+di*2+dc)]
    # slot = 4 rows (y = 2rw-1+di, di 0..3) x 2 cols (x = 2cp-1+dc, dc 0..1):
    # the 4-row window equals the conv2 tile's full RF height, so each
    # standard conv2 tile contracts just 2 slots (left/right col pair)
    w1n = np.zeros((128, NSLOT, 128), np.float32)
    for rw in range(5):
        for cp in range(6):
            s = cp * 5 + rw
            for ci in range(CH1):
                for di in range(4):
                    y = 2 * rw - 1 + di
                    if not (0 <= y < HH):
                        continue
                    for dc in range(2):
                        x = 2 * cp - 1 + dc
                        if not (0 <= x < WW):
                            continue
                        p = ci * 8 + di * 2 + dc
                        for ky in range(3):
                            yy = y + ky - 1
                            if not (0 <= yy < HH):
                                continue
                            for kx in range(3):
                                xx = x + kx - 1
                                if not (0 <= xx < WW):
                                    continue
                                w1n[yy * WW + xx, s, p] = conv1_w[ci, 0, ky, kx]

    # conv2 lhsT (4 shared matrices):
    #  h=0: std left pair (cp=jp):  dy = di-1-mi, dx = dc-1-mj
    #  h=1: std right pair (cp=jp+1): dy = di-1-mi, dx = dc+1-mj
    #  h=2: j=10 J0 (aligned window, all rows): dy = di-1-r, dx = dc-1
    #  h=3: j=10 J1 (next window, top rows only, di>=2): dy = di+1-r
    w2n = np.zeros((128, 4, 128), np.float32)
    for ci in range(CH1):
        for di in range(4):
            for dc in range(2):
                p = ci * 8 + di * 2 + dc
                for co in range(CH2):
                    for mi in range(2):
                        for mj in range(2):
                            m = co * 4 + mi * 2 + mj
                            dy = di - 1 - mi
                            if -1 <= dy <= 1:
                                dx = dc - 1 - mj
                                if -1 <= dx <= 1:
                                    w2n[p, 0, m] = conv2_w[co, ci, dy + 1, dx + 1]
                                dx = dc + 1 - mj
                                if -1 <= dx <= 1:
                                    w2n[p, 1, m] = conv2_w[co, ci, dy + 1, dx + 1]
                    for r in range(4):
                        m = co * 4 + r
                        dx = dc - 1
                        if dx != -1 and dx != 0:
                            continue
                        dy = di - 1 - r
                        if -1 <= dy <= 1:
                            w2n[p, 2, m] = conv2_w[co, ci, dy + 1, dx + 1]
                        dy = di + 1 - r
                        if di >= 2 and -1 <= dy <= 1:
                            w2n[p, 3, m] = conv2_w[co, ci, dy + 1, dx + 1]

    # fc3 lhsT chunks matching F layout:
    #   k = jp*5 + ip (jp 0..4): m = co*4+mi*2+mj -> (2ip+mi, 2jp+mj)
    #   k = 25,26,27: j=10 tiles rows base 0/4/8: m = co*4+r -> (base+r, 10)
    fc3t = np.zeros((NK, 128, 256), np.float32)
    for jp in range(5):
        for ip in range(5):
            k = jp * 5 + ip
            for m in range(128):
                co, rem = divmod(m, 4)
                mi, mj = divmod(rem, 2)
                i = 2 * ip + mi
                j = 2 * jp + mj
                fc3t[k, m, :] = fc3_w[:, co * SP + i * WW + j]
    for b, base in enumerate((0, 4, 8)):
        k = 25 + b
        for m in range(128):
            co, r = divmod(m, 4)
            i = base + r
            if i < HH:
                fc3t[k, m, :] = fc3_w[:, co * SP + i * WW + 10]

    wiht = np.ascontiguousarray(
        w_ih.reshape(2 * HID, 2, 128).transpose(1, 2, 0)
    )  # [mf, p, gate]
    whht = np.ascontiguousarray(w_hh.T).copy()  # [64, 128]
    whht[:, :HID] *= 0.5  # forget-gate pre-scale (fg = gif' + 0.5*ghf)
    fc5t = np.ascontiguousarray(fc5_w.T)  # [64, 7]
    w1b = w1n.reshape(128, NSLOT * 128).astype(BF16NP)
    w2b = w2n.reshape(128, 4 * 128).astype(BF16NP)
    f3b = fc3t.reshape(NK * 128, 256).astype(BF16NP)
    f3p = fc3t.transpose(1, 0, 2).reshape(128, NK * 256)
    boot = np.concatenate(
        [w1n.reshape(128, NSLOT * 128)[:, : 10 * 128],
         w2n.reshape(128, 4 * 128), f3p[:, : 7 * 256]], axis=1
    ).astype(BF16NP)
    return (
        boot,
        w1b,
        w2b,
        f3b,
        wiht.reshape(2 * 128, 128).astype(BF16NP),
        np.ascontiguousarray(whht, np.float32),  # f32r on device
        fc5t.astype(BF16NP),
    )


def _pack_x(x_shard):
    # host-side quantize (exact: k/128 values are exact in bf16), then
    # [BS, T, HH, WW] -> [110, S] with s = t*BS + b
    BS = x_shard.shape[0]
    xq = np.round(np.clip(x_shard, -1.0, 1.0) * 128.0).astype(np.float32) / 128.0
    xt = xq.transpose(1, 0, 2, 3).reshape(T * BS, SP).T
    return np.ascontiguousarray(xt).astype(BF16NP)


def _relu_safe(x, conv1_w, conv2_w):
    """True if conv1/conv2 pre-activations never exceed +1 for this data, so
    clip(v,0,1) == relu(v) and the epilogues can use single-op Relu."""
    xq = np.round(np.clip(x, -1.0, 1.0) * 128.0) / 128.0
    B = x.shape[0] * x.shape[1]
    xp = np.zeros((B, HH + 2, WW + 2), np.float32)
    xp[:, 1:-1, 1:-1] = xq.reshape(B, HH, WW)
    z1 = np.zeros((B, CH1, HH, WW), np.float32)
    for ky in range(3):
        for kx in range(3):
            z1 += (
                xp[:, None, ky : ky + HH, kx : kx + WW]
                * conv1_w[None, :, 0, ky, kx, None, None]
            )
    if z1.max() >= 0.995:
        return False
    a1 = np.clip(z1, 0.0, 1.0)
    a1p = np.zeros((B, CH1, HH + 2, WW + 2), np.float32)
    a1p[:, :, 1:-1, 1:-1] = a1
    z2 = np.zeros((B, CH2, HH, WW), np.float32)
    for ky in range(3):
        for kx in range(3):
            z2 += np.einsum(
                "bcyx,oc->boyx",
                a1p[:, :, ky : ky + HH, kx : kx + WW],
                conv2_w[:, :, ky, kx],
                optimize=True,
            )
    return z2.max() < 0.995


# ---------------------------------------------------------------- bass builder
def build_nc(BS=256, relu_acts=False, wpcs=(1, 2, 2, 2, 2, 1)):
    S = T * BS
    assert sum(wpcs) == T
    nc = bacc.Bacc()

    xt_d = nc.declare_dram_parameter("xt", [SP, S], BF16, isOutput=False)
    bt_d = nc.declare_dram_parameter("boot", [128, 3584], BF16, isOutput=False)
    w1_d = nc.declare_dram_parameter("w1b", [128, NSLOT * 128], BF16, isOutput=False)
    w2_d = nc.declare_dram_parameter("w2b", [128, 4 * 128], BF16, isOutput=False)
    f3_d = nc.declare_dram_parameter("fc3t", [NK * 128, 256], BF16, isOutput=False)
    wi_d = nc.declare_dram_parameter("wiht", [2 * 128, 128], BF16, isOutput=False)
    wh_d = nc.declare_dram_parameter("whht", [HID, 128], F32R, isOutput=False)
    f5_d = nc.declare_dram_parameter("fc5t", [HID, NCLS], BF16, isOutput=False)
    out_d = nc.declare_dram_parameter("out", [NCLS, BS], F32, isOutput=True)

    MX = mybir.AluOpType.max
    MN = mybir.AluOpType.min
    AD = mybir.AluOpType.add
    SU = mybir.AluOpType.subtract
    MU = mybir.AluOpType.mult

    with tile.TileContext(nc) as tc:
        with (
            tc.tile_pool(name="static", bufs=1) as st,
            tc.tile_pool(name="a1p", bufs=2) as a1p,
            tc.tile_pool(name="fp", bufs=2) as fp,
            tc.tile_pool(name="a3p", bufs=2) as a3p,
            tc.tile_pool(name="rp", bufs=2) as rp,
            tc.tile_pool(name="c1ps", bufs=3, space="PSUM") as c1ps,
            tc.tile_pool(name="c2ps", bufs=2, space="PSUM") as c2ps,
            tc.tile_pool(name="f3ps", bufs=2, space="PSUM") as f3ps,
            tc.tile_pool(name="mgups", bufs=1, space="PSUM") as mgups,
        ):
            # ---- static loads, interleaved so the column-major conv/fc3
            # stream of chunk 0 never waits: per column jp it needs W1[cw=jp],
            # W2, and FC3 k-chunks 5jp..5jp+4
            XQ = st.tile([SP, S], BF16)
            off0 = wpcs[0] * BS
            nc.sync.dma_start(XQ[:, 0:off0], xt_d.ap()[:, 0:off0])
            # chunk-0-critical weights (w1 slots 0-9 | w2 | fc3 k 0-6) ride
            # ONE boot DMA: the SP descriptor-gen serialization (~1.2us per
            # dma_start) is the startup critical path
            BOOT = st.tile([128, 3584], BF16)
            nc.sync.dma_start(BOOT[:, 0:1792], bt_d.ap()[:, 0:1792])
            nc.sync.dma_start(BOOT[:, 1792:3584], bt_d.ap()[:, 1792:3584])
            W1 = st.tile([128, NSLOT, 128], BF16)
            w1_ap = w1_d.ap().rearrange("k (s p) -> k s p", s=NSLOT)
            FC3 = st.tile([128, NK, 256], BF16)
            f3_ap = f3_d.ap().rearrange("(k p) f -> p k f", k=NK)
            # remaining loads in deadline order
            nc.sync.dma_start(FC3[:, 7:14, :], f3_ap[:, 7:14, :])
            nc.sync.dma_start(W1[:, 10:20, :], w1_ap[:, 10:20, :])
            nc.sync.dma_start(FC3[:, 14:21, :], f3_ap[:, 14:21, :])
            nc.sync.dma_start(W1[:, 20:30, :], w1_ap[:, 20:30, :])
            nc.sync.dma_start(FC3[:, 21:28, :], f3_ap[:, 21:28, :])

            def w1v(s):
                if s < 10:
                    return BOOT[:SP, s * 128 : (s + 1) * 128]
                return W1[:SP, s, :]

            def w2v(h):
                return BOOT[:, 1280 + h * 128 : 1280 + (h + 1) * 128]

            def f3v(k, mf):
                if k < 7:
                    off = 1792 + k * 256 + mf * 128
                    return BOOT[:, off : off + 128]
                return FC3[:, k, bass.ts(mf, 128)]
            nc.sync.dma_start(XQ[:, off0:S], xt_d.ap()[:, off0:S])
            WIH = st.tile([128, 2, 128], BF16)
            nc.sync.dma_start(WIH[:], wi_d.ap().rearrange("(m p) g -> p m g", m=2))
            WHH = st.tile([HID, 128], F32R)
            nc.sync.dma_start(WHH[:], wh_d.ap())
            FC5 = st.tile([HID, NCLS], BF16)
            nc.sync.dma_start(FC5[:], f5_d.ap())

            GIF = st.tile([HID, S], F32)  # 0.5*gif + 0.5
            GIN = st.tile([HID, S], F32)  # gin

            H = st.tile([HID, BS], F32)
            HF = st.tile([HID, BS], BF16)  # final hidden for fc5
            nc.vector.memset(H[:], 0.0)

            # ---------------- epilogue helper (PSUM -> SBUF + relu/clip)
            def epilogue(dst, ps, use_act):
                if relu_acts:
                    if use_act:
                        nc.scalar.activation(
                            dst, ps, mybir.ActivationFunctionType.Relu
                        )
                    else:
                        nc.vector.tensor_scalar_max(dst, ps, 0.0)
                else:
                    nc.vector.tensor_scalar(dst, ps, 0.0, 1.0, MX, MN)

            # ---------------- MGU window emitter (DVE chain + gh/fc5 MMs)
            # returns a GENERATOR: the chunk loop pumps a few ops at a time
            # between conv columns, so each chain op's ~250ns semaphore wait
            # sits in the engine wait-queue while epilogues execute behind it
            def mgu_window(t, parts=1):
                def emit():
                    qtmp = rp.tile([HID, BS], F32, name="qtmp")
                    HQ = rp.tile([HID, BS], F32R, name="HQ")
                    fg = rp.tile([HID, BS], F32, name="fg")
                    ng = rp.tile([HID, BS], F32, name="ng")
                    dt = rp.tile([HID, BS], F32, name="dt")
                    Hdst = HF[:] if t == T - 1 else H[:]
                    cs = slice(0, BS)
                    gsl = bass.ds(t * BS, BS)
                    # hq = round(h*128)/128 via magic-number trick; ACT's
                    # scale/bias path (s*x+b in fp32) does each step in
                    # one op and keeps the serial chain off the DVE
                    nc.scalar.activation(
                        qtmp[:, cs], H[:, cs],
                        mybir.ActivationFunctionType.Copy,
                        bias=MAGIC, scale=128.0,
                    )
                    nc.scalar.activation(
                        HQ[:, cs], qtmp[:, cs],
                        mybir.ActivationFunctionType.Copy,
                        bias=-MAGIC * INV_SCALE, scale=INV_SCALE,
                    )
                    # gh = whh'.T @ hq: one M=128 matmul; f=0:64, n=64:128
                    psg = mgups.tile([128, BS], F32, name="psg")
                    nc.tensor.matmul(
                        psg[:], WHH[:], HQ[:, cs], start=True, stop=True
                    )
                    yield
                    # fg = clip(gif' + 0.5*ghf, 0, 1)
                    nc.vector.tensor_tensor(
                        fg[:, cs], GIF[:, gsl], psg[0:HID, :], AD
                    )
                    yield
                    nc.vector.tensor_scalar(fg[:, cs], fg[:, cs], 0.0, 1.0, MX, MN)
                    yield
                    # ng = clip(gin + fg*ghn, -1, 1)
                    nc.vector.tensor_tensor(
                        ng[:, cs], fg[:, cs], psg[HID:128, :], MU
                    )
                    yield
                    nc.vector.tensor_tensor(ng[:, cs], ng[:, cs], GIN[:, gsl], AD)
                    yield
                    nc.vector.tensor_scalar(ng[:, cs], ng[:, cs], -1.0, 1.0, MX, MN)
                    yield
                    # h' = ng + fg*(hq - ng)
                    nc.vector.tensor_tensor(
                        dt[:, cs], HQ[:, cs].bitcast(F32), ng[:, cs], SU
                    )
                    yield
                    nc.vector.tensor_tensor(dt[:, cs], dt[:, cs], fg[:, cs], MU)
                    yield
                    nc.vector.tensor_tensor(Hdst[:, cs], ng[:, cs], dt[:, cs], AD)
                    if t == T - 1:
                        pso = mgups.tile([NCLS, BS], F32, name="pso", tag="psg")
                        nc.tensor.matmul(
                            pso[:], FC5[:], HF[:, cs], start=True, stop=True
                        )
                        OUTS = rp.tile([NCLS, BS], F32, name="OUTS", bufs=1)
                        nc.vector.tensor_copy(OUTS[:], pso[:])
                        nc.sync.dma_start(out_d.ap(), OUTS[:])
                return emit

            # ---------------- encoder chunks
            pending = []  # deferred MGU op-generators from the previous chunk
            pending_gi = []  # deferred gi matmuls from the previous chunk

            def pump(n):
                while n > 0 and pending:
                    try:
                        next(pending[0])
                        n -= 1
                    except StopIteration:
                        pending.pop(0)

            def drain():
                while pending:
                    try:
                        next(pending[0])
                    except StopIteration:
                        pending.pop(0)
            col = 0
            t0 = 0
            for u, wpc in enumerate(wpcs):
                Sc = wpc * BS
                sl = bass.ds(col, Sc)
                last = u == len(wpcs) - 1
                A1 = a1p.tile([128, NSLOT, Sc], BF16, name="A1")
                F = fp.tile([128, NK, Sc], BF16, name="F")
                A3 = a3p.tile([128, 2, Sc], BF16, name="A3")
                psmf = [
                    f3ps.tile([128, Sc], F32, name=f"psmf{mf}", tag="ps3")
                    for mf in range(2)
                ]
                c1n = 0
                c2n = 0
                fc3_started = [False]

                def conv1_slot(s):
                    nonlocal c1n
                    ps1 = c1ps.tile([128, Sc], F32, name="ps1")
                    nc.tensor.matmul(
                        ps1[:], w1v(s), XQ[:, sl], start=True, stop=True
                    )
                    # chunk 0 cold-start: ACT is busy with table load + DMA
                    # issue, so the first column's epilogues go to DVE
                    pat = [False] * 5 if (u == 0 and c1n < 5) else C1_PAT[wpc]
                    epilogue(A1[:, s, :], ps1[:], pat[c1n % len(pat)])
                    c1n += 1

                def conv2_mms(tiles):
                    # tiles: list of (k, [(h, s), ...]) accumulation specs,
                    # emitted h-major for lhsT reuse
                    nonlocal c2n
                    pss = {}
                    nacc = {k: len(accs) for k, accs in tiles}
                    cnt = {k: 0 for k, _ in tiles}
                    for qi in range(max(nacc.values())):
                        for k, accs in tiles:
                            if qi >= len(accs):
                                continue
                            h, s = accs[qi]
                            if k not in pss:
                                pss[k] = c2ps.tile([128, Sc], F32, name="ps2")
                            nc.tensor.matmul(
                                pss[k][:],
                                w2v(h),
                                A1[:, s, :],
                                start=(cnt[k] == 0),
                                stop=(cnt[k] == nacc[k] - 1),
                                skip_group_check=True,
                            )
                            cnt[k] += 1
                    for k, _ in tiles:
                        pat = C2_PAT[wpc]
                        epilogue(F[:, k, :], pss[k][:], pat[c2n % len(pat)])
                        c2n += 1
                        for mf in range(2):
                            nc.tensor.matmul(
                                psmf[mf][:],
                                f3v(k, mf),
                                F[:, k, :],
                                start=(not fc3_started[0]),
                                stop=(k == NK - 1),
                                skip_group_check=True,
                            )
                        fc3_started[0] = True

                for cp in range(6):
                    for rw in range(5):
                        conv1_slot(cp * 5 + rw)
                    if cp == 0 and pending_gi:
                        pending_gi.pop(0)()
                    if cp >= 1:
                        jp = cp - 1  # std tiles using col pairs (jp, jp+1)
                        for ip0 in (0, 2, 4):
                            tl = [
                                (
                                    jp * 5 + ip,
                                    [(0, jp * 5 + ip), (1, cp * 5 + ip)],
                                )
                                for ip in (ip0, ip0 + 1)
                                if ip < 5
                            ]
                            conv2_mms(tl)
                    if cp == 5:
                        b = 25  # cp=5 slot base; J0=h2, J1=h3
                        conv2_mms(
                            [
                                (25, [(2, b + 0), (3, b + 1)]),
                                (26, [(2, b + 2), (3, b + 3)]),
                            ]
                        )
                        conv2_mms([(27, [(2, b + 4)])])
                    # emit deferred MGU windows as whole blocks mid-chunk
                    if cp == 1 or cp == 3:
                        pump(12)
                drain()

                # A3 = clip(fc3 psum, 0, 1)
                for mf in range(2):
                    nc.vector.tensor_scalar(
                        A3[:, mf, :], psmf[mf][:], 0.0, 1.0, MX, MN
                    )

                # gi: single M=128 accumulation pair; deferred into the next
                # chunk's PE stream so the A3-epilogue wait never stalls PE
                def gi_emit(A3=A3, sl=sl, Sc=Sc):
                    psgi = mgups.tile([128, Sc], F32, name="psgi", tag="psg")
                    for mf in range(2):
                        nc.tensor.matmul(
                            psgi[:],
                            WIH[:, mf, :],
                            A3[:, mf, :],
                            start=(mf == 0),
                            stop=(mf == 1),
                            skip_group_check=True,
                        )
                    nc.vector.tensor_scalar(
                        GIF[:, sl], psgi[0:HID, :], 0.5, 0.5, MU, AD
                    )
                    nc.vector.tensor_copy(GIN[:, sl], psgi[HID:128, :])

                if last:
                    gi_emit()
                else:
                    pending_gi.append(gi_emit)

                for w in range(wpc):
                    t = t0 + w
                    pending.append(mgu_window(t, 1)())
                if last:
                    drain()
                t0 += wpc
                col += Sc

    nc.compile()
    return nc


def _weight_map(packs):
    boot, w1b, w2b, fc3t, wiht, whht, fc5t = packs
    return {
        "boot": boot, "w1b": w1b, "w2b": w2b,
        "fc3t": fc3t, "wiht": wiht, "whht": whht, "fc5t": fc5t,
    }


# ---------------------------------------------------------------- entry point
def kernel(**inputs):
    x = np.asarray(inputs["x"], np.float32)
    packs = _pack_weights(
        np.asarray(inputs["conv1_w"], np.float32),
        np.asarray(inputs["conv2_w"], np.float32),
        np.asarray(inputs["fc3_w"], np.float32),
        np.asarray(inputs["w_ih"], np.float32),
        np.asarray(inputs["w_hh"], np.float32),
        np.asarray(inputs["fc5_w"], np.float32),
    )
    NCORES = 8
    B = x.shape[0]
    BS = B // NCORES

    relu_ok = _relu_safe(
        x, np.asarray(inputs["conv1_w"], np.float32),
        np.asarray(inputs["conv2_w"], np.float32),
    )
    nc = build_nc(BS=BS, relu_acts=relu_ok)
    in_maps = [dict(_weight_map(packs), xt=_pack_x(x[c * BS : (c + 1) * BS]))
               for c in range(NCORES)]
    res = run_bass_kernel_spmd(nc, in_maps, core_ids=list(range(NCORES)))
    out = np.concatenate([res.results[c]["out"].T for c in range(NCORES)], axis=0)
    return np.ascontiguousarray(out, np.float32)


if __name__ == "__main__":
    rng = np.random.default_rng(0)
    ins = {
        "x": rng.standard_normal((2048, T, HH, WW), np.float32) * 0.5,
        "conv1_w": rng.standard_normal((CH1, 1, 3, 3), np.float32) * 0.1,
        "conv2_w": rng.standard_normal((CH2, CH1, 3, 3), np.float32) * 0.1,
        "fc3_w": rng.standard_normal((256, 3520), np.float32) * 0.1,
        "w_ih": rng.standard_normal((128, 256), np.float32) * 0.1,
        "w_hh": rng.standard_normal((128, HID), np.float32) * 0.1,
        "fc5_w": rng.standard_normal((NCLS, HID), np.float32) * 0.1,
    }
    out = kernel(**ins)
    print(out.shape, out.dtype, np.abs(out).mean())


# revision 73
# speedup vs baseline: 1.0137x; 1.0137x over previous
"""Trainium2 Bass kernel for nn_Cascade_CNN_RNN (cascade CNN -> MGU RNN).

Data-parallel over batch across 8 NeuronCores. Per core (shard B=256):
  - x quantized on HOST (round-half-even to k/128, exact in bf16) and
    uploaded as bf16 [110, T*BS]
  - conv1 as banded spatial-operator matmuls -> a1 in 30 slots of
    4 rows x 2 cols [p = ci*8 + di*2 + dc], slot s = cp*5 + rw; the
    4-row window equals the conv2 tile RF height so each standard conv2
    tile contracts exactly 2 slots with 2 shared lhsT
  - conv2: 25 standard 2x2 tiles (2 K-slots) + 3 j=10 tiles (5 MMs
    total, 2 more shared lhsT) -> F [128, 28, Sc]
  - fc3 28 K-chunks + gi (single M=128 accumulation pair), all
    column-interleaved with the conv stream so every engine pipelines;
    epilogues split ACT/DVE by static patterns
  - gi and MGU windows deferred into the NEXT chunk's PE stream; gh is
    one M=128 matmul read by DVE at PSUM partition offsets 0/64; hq
    magic-round quantize on the ACT engine (fp32-exact scale+bias)
  - chunks of [1,2,2,2,2,1] windows (small first chunk hides in the
    DMA/preamble ramp); tail = window 9 chain + fc5
All matmul-facing tensors are bf16 (full-speed PE + FWL weight loads);
PSUM accumulation is fp32, MGU recurrence state fp32 (whh as f32r).
"""

import numpy as np
import ml_dtypes

import concourse.bass as bass
import concourse.mybir as mybir
import concourse.tile as tile
from concourse import bacc
from concourse.bass_utils import run_bass_kernel_spmd

F32 = mybir.dt.float32
F32R = mybir.dt.float32r
BF16 = mybir.dt.bfloat16
BF16NP = ml_dtypes.bfloat16
MAGIC = 12582912.0  # 1.5 * 2**23: fp32 round-to-nearest-even integer trick
INV_SCALE = 0.0078125  # 1/128

T, HH, WW = 10, 10, 11  # windows, height, width
SP = HH * WW  # 110 input spatial positions
CH1, CH2 = 16, 32
NCLS = 7
HID = 64

NSLOT = 30   # conv1 slots: s = cp*5 + rw (col-pair major, 4x2 positions)
NK = 28      # fc3 K-chunks / conv2 output tiles

# epilogue engine split patterns (True -> ACT/scalar, False -> DVE),
# keyed by windows-per-chunk (small chunks are elementwise-bound, so the
# DVE keeps a lighter share there)
C1_PAT = {2: [True, True, False], 1: [True, True, True, False]}
C2_PAT = {2: [True, False], 1: [True, False]}


# ---------------------------------------------------------------- host packing
def _pack_weights(conv1_w, conv2_w, fc3_w, w_ih, w_hh, fc5_w):
    # conv1 operator lhsT: [k=(yy*11+xx), slot=(cp*5+rw), p=(ci*8+di*2+dc)]
    # slot = 4 rows (y = 2rw-1+di, di 0..3) x 2 cols (x = 2cp-1+dc, dc 0..1):
    # the 4-row window equals the conv2 tile's full RF height, so each
    # standard conv2 tile contracts just 2 slots (left/right col pair)
    w1n = np.zeros((128, NSLOT, 128), np.float32)
    for rw in range(5):
        for cp in range(6):
            s = cp * 5 + rw
            for ci in range(CH1):
                for di in range(4):
                    y = 2 * rw - 1 + di
                    if not (0 <= y < HH):
                        continue
                    for dc in range(2):
                        x = 2 * cp - 1 + dc
                        if not (0 <= x < WW):
                            continue
                        p = ci * 8 + di * 2 + dc
                        for ky in range(3):
                            yy = y + ky - 1
                            if not (0 <= yy < HH):
                                continue
                            for kx in range(3):
                                xx = x + kx - 1
                                if not (0 <= xx < WW):
                                    continue
                                w1n[yy * WW + xx, s, p] = conv1_w[ci, 0, ky, kx]

    # conv2 lhsT (4 shared matrices):
    #  h=0: std left pair (cp=jp):  dy = di-1-mi, dx = dc-1-mj
    #  h=1: std right pair (cp=jp+1): dy = di-1-mi, dx = dc+1-mj
    #  h=2: j=10 J0 (aligned window, all rows): dy = di-1-r, dx = dc-1
    #  h=3: j=10 J1 (next window, top rows only, di>=2): dy = di+1-r
    w2n = np.zeros((128, 4, 128), np.float32)
    for ci in range(CH1):
        for di in range(4):
            for dc in range(2):
                p = ci * 8 + di * 2 + dc
                for co in range(CH2):
                    for mi in range(2):
                        for mj in range(2):
                            m = co * 4 + mi * 2 + mj
                            dy = di - 1 - mi
                            if -1 <= dy <= 1:
                                dx = dc - 1 - mj
                                if -1 <= dx <= 1:
                                    w2n[p, 0, m] = conv2_w[co, ci, dy + 1, dx + 1]
                                dx = dc + 1 - mj
                                if -1 <= dx <= 1:
                                    w2n[p, 1, m] = conv2_w[co, ci, dy + 1, dx + 1]
                    for r in range(4):
                        m = co * 4 + r
                        dx = dc - 1
                        if dx != -1 and dx != 0:
                            continue
                        dy = di - 1 - r
                        if -1 <= dy <= 1:
                            w2n[p, 2, m] = conv2_w[co, ci, dy + 1, dx + 1]
                        dy = di + 1 - r
                        if di >= 2 and -1 <= dy <= 1:
                            w2n[p, 3, m] = conv2_w[co, ci, dy + 1, dx + 1]

    # fc3 lhsT chunks matching F layout:
    #   k = jp*5 + ip (jp 0..4): m = co*4+mi*2+mj -> (2ip+mi, 2jp+mj)
    #   k = 25,26,27: j=10 tiles rows base 0/4/8: m = co*4+r -> (base+r, 10)
    fc3t = np.zeros((NK, 128, 256), np.float32)
    for jp in range(5):
        for ip in range(5):
            k = jp * 5 + ip
            for m in range(128):
                co, rem = divmod(m, 4)
                mi, mj = divmod(rem, 2)
                i = 2 * ip + mi
                j = 2 * jp + mj
                fc3t[k, m, :] = fc3_w[:, co * SP + i * WW + j]
    for b, base in enumerate((0, 4, 8)):
        k = 25 + b
        for m in range(128):
            co, r = divmod(m, 4)
            i = base + r
            if i < HH:
                fc3t[k, m, :] = fc3_w[:, co * SP + i * WW + 10]

    wiht = np.ascontiguousarray(
        w_ih.reshape(2 * HID, 2, 128).transpose(1, 2, 0)
    )  # [mf, p, gate]
    whht = np.ascontiguousarray(w_hh.T).copy()  # [64, 128]
    whht[:, :HID] *= 0.5  # forget-gate pre-scale (fg = gif' + 0.5*ghf)
    fc5t = np.ascontiguousarray(fc5_w.T)  # [64, 7]
    return (
        w1n.reshape(128, NSLOT * 128).astype(BF16NP),
        w2n.reshape(128, 4 * 128).astype(BF16NP),
        fc3t.reshape(NK * 128, 256).astype(BF16NP),
        wiht.reshape(2 * 128, 128).astype(BF16NP),
        np.ascontiguousarray(whht, np.float32),  # f32r on device
        fc5t.astype(BF16NP),
    )


def _pack_x(x_shard):
    # host-side quantize (exact: k/128 values are exact in bf16), then
    # [BS, T, HH, WW] -> [110, S] with s = t*BS + b
    BS = x_shard.shape[0]
    xq = np.round(np.clip(x_shard, -1.0, 1.0) * 128.0).astype(np.float32) / 128.0
    xt = xq.transpose(1, 0, 2, 3).reshape(T * BS, SP).T
    return np.ascontiguousarray(xt).astype(BF16NP)


def _relu_safe(x, conv1_w, conv2_w):
    """True if conv1/conv2 pre-activations never exceed +1 for this data, so
    clip(v,0,1) == relu(v) and the epilogues can use single-op Relu."""
    xq = np.round(np.clip(x, -1.0, 1.0) * 128.0) / 128.0
    B = x.shape[0] * x.shape[1]
    xp = np.zeros((B, HH + 2, WW + 2), np.float32)
    xp[:, 1:-1, 1:-1] = xq.reshape(B, HH, WW)
    z1 = np.zeros((B, CH1, HH, WW), np.float32)
    for ky in range(3):
        for kx in range(3):
            z1 += (
                xp[:, None, ky : ky + HH, kx : kx + WW]
                * conv1_w[None, :, 0, ky, kx, None, None]
            )
    if z1.max() >= 0.995:
        return False
    a1 = np.clip(z1, 0.0, 1.0)
    a1p = np.zeros((B, CH1, HH + 2, WW + 2), np.float32)
    a1p[:, :, 1:-1, 1:-1] = a1
    z2 = np.zeros((B, CH2, HH, WW), np.float32)
    for ky in range(3):
        for kx in range(3):
            z2 += np.einsum(
                "bcyx,oc->boyx",
                a1p[:, :, ky : ky + HH, kx : kx + WW],
                conv2_w[:, :, ky, kx],
                optimize=True,
            )
    return z2.max() < 0.995


# ---------------------------------------------------------------- bass builder
def build_nc(BS=256, relu_acts=False, wpcs=(1, 2, 2, 2, 2, 1)):
    S = T * BS
    assert sum(wpcs) == T
    nc = bacc.Bacc()

    xt_d = nc.declare_dram_parameter("xt", [SP, S], BF16, isOutput=False)
    w1_d = nc.declare_dram_parameter("w1b", [128, NSLOT * 128], BF16, isOutput=False)
    w2_d = nc.declare_dram_parameter("w2b", [128, 4 * 128], BF16, isOutput=False)
    f3_d = nc.declare_dram_parameter("fc3t", [NK * 128, 256], BF16, isOutput=False)
    wi_d = nc.declare_dram_parameter("wiht", [2 * 128, 128], BF16, isOutput=False)
    wh_d = nc.declare_dram_parameter("whht", [HID, 128], F32R, isOutput=False)
    f5_d = nc.declare_dram_parameter("fc5t", [HID, NCLS], BF16, isOutput=False)
    out_d = nc.declare_dram_parameter("out", [NCLS, BS], F32, isOutput=True)

    MX = mybir.AluOpType.max
    MN = mybir.AluOpType.min
    AD = mybir.AluOpType.add
    SU = mybir.AluOpType.subtract
    MU = mybir.AluOpType.mult

    with tile.TileContext(nc) as tc:
        with (
            tc.tile_pool(name="static", bufs=1) as st,
            tc.tile_pool(name="a1p", bufs=2) as a1p,
            tc.tile_pool(name="fp", bufs=2) as fp,
            tc.tile_pool(name="a3p", bufs=2) as a3p,
            tc.tile_pool(name="rp", bufs=2) as rp,
            tc.tile_pool(name="c1ps", bufs=3, space="PSUM") as c1ps,
            tc.tile_pool(name="c2ps", bufs=2, space="PSUM") as c2ps,
            tc.tile_pool(name="f3ps", bufs=2, space="PSUM") as f3ps,
            tc.tile_pool(name="mgups", bufs=1, space="PSUM") as mgups,
        ):
            # ---- static loads, interleaved so the column-major conv/fc3
            # stream of chunk 0 never waits: per column jp it needs W1[cw=jp],
            # W2, and FC3 k-chunks 5jp..5jp+4
            XQ = st.tile([SP, S], BF16)
            off0 = wpcs[0] * BS
            nc.sync.dma_start(XQ[:, 0:off0], xt_d.ap()[:, 0:off0])
            W1 = st.tile([128, NSLOT, 128], BF16)
            w1_ap = w1_d.ap().rearrange("k (s p) -> k s p", s=NSLOT)
            nc.sync.dma_start(W1[:, 0:10, :], w1_ap[:, 0:10, :])
            W2 = st.tile([128, 4, 128], BF16)
            nc.sync.dma_start(W2[:], w2_d.ap().rearrange("p (h m) -> p h m", h=4))
            FC3 = st.tile([128, NK, 256], BF16)
            f3_ap = f3_d.ap().rearrange("(k p) f -> p k f", k=NK)
            nc.sync.dma_start(FC3[:, 0:7, :], f3_ap[:, 0:7, :])
            # remaining loads in deadline order (SP descriptor-gen is ~1.2us
            # per dma_start, so issue count and order both matter)
            nc.sync.dma_start(FC3[:, 7:14, :], f3_ap[:, 7:14, :])
            nc.sync.dma_start(W1[:, 10:20, :], w1_ap[:, 10:20, :])
            nc.sync.dma_start(FC3[:, 14:21, :], f3_ap[:, 14:21, :])
            nc.sync.dma_start(W1[:, 20:30, :], w1_ap[:, 20:30, :])
            nc.sync.dma_start(FC3[:, 21:28, :], f3_ap[:, 21:28, :])
            nc.sync.dma_start(XQ[:, off0:S], xt_d.ap()[:, off0:S])
            WIH = st.tile([128, 2, 128], BF16)
            nc.sync.dma_start(WIH[:], wi_d.ap().rearrange("(m p) g -> p m g", m=2))
            WHH = st.tile([HID, 128], F32R)
            nc.sync.dma_start(WHH[:], wh_d.ap())
            FC5 = st.tile([HID, NCLS], BF16)
            nc.sync.dma_start(FC5[:], f5_d.ap())

            GIF = st.tile([HID, S], F32)  # 0.5*gif + 0.5
            GIN = st.tile([HID, S], F32)  # gin

            H = st.tile([HID, BS], F32)
            HF = st.tile([HID, BS], BF16)  # final hidden for fc5
            nc.vector.memset(H[:], 0.0)

            # ---------------- epilogue helper (PSUM -> SBUF + relu/clip)
            def epilogue(dst, ps, use_act):
                if relu_acts:
                    if use_act:
                        nc.scalar.activation(
                            dst, ps, mybir.ActivationFunctionType.Relu
                        )
                    else:
                        nc.vector.tensor_scalar_max(dst, ps, 0.0)
                else:
                    nc.vector.tensor_scalar(dst, ps, 0.0, 1.0, MX, MN)

            # ---------------- MGU window emitter (DVE chain + gh/fc5 MMs)
            # returns a GENERATOR: the chunk loop pumps a few ops at a time
            # between conv columns, so each chain op's ~250ns semaphore wait
            # sits in the engine wait-queue while epilogues execute behind it
            def mgu_window(t, parts=1):
                def emit():
                    qtmp = rp.tile([HID, BS], F32, name="qtmp")
                    HQ = rp.tile([HID, BS], F32R, name="HQ")
                    fg = rp.tile([HID, BS], F32, name="fg")
                    ng = rp.tile([HID, BS], F32, name="ng")
                    dt = rp.tile([HID, BS], F32, name="dt")
                    Hdst = HF[:] if t == T - 1 else H[:]
                    cs = slice(0, BS)
                    gsl = bass.ds(t * BS, BS)
                    # hq = round(h*128)/128 via magic-number trick; ACT's
                    # scale/bias path (s*x+b in fp32) does each step in
                    # one op and keeps the serial chain off the DVE
                    nc.scalar.activation(
                        qtmp[:, cs], H[:, cs],
                        mybir.ActivationFunctionType.Copy,
                        bias=MAGIC, scale=128.0,
                    )
                    nc.scalar.activation(
                        HQ[:, cs], qtmp[:, cs],
                        mybir.ActivationFunctionType.Copy,
                        bias=-MAGIC * INV_SCALE, scale=INV_SCALE,
                    )
                    # gh = whh'.T @ hq: one M=128 matmul; f=0:64, n=64:128
                    psg = mgups.tile([128, BS], F32, name="psg")
                    nc.tensor.matmul(
                        psg[:], WHH[:], HQ[:, cs], start=True, stop=True
                    )
                    yield
                    # fg = clip(gif' + 0.5*ghf, 0, 1)
                    nc.vector.tensor_tensor(
                        fg[:, cs], GIF[:, gsl], psg[0:HID, :], AD
                    )
                    yield
                    nc.vector.tensor_scalar(fg[:, cs], fg[:, cs], 0.0, 1.0, MX, MN)
                    yield
                    # ng = clip(gin + fg*ghn, -1, 1)
                    nc.vector.tensor_tensor(
                        ng[:, cs], fg[:, cs], psg[HID:128, :], MU
                    )
                    yield
                    nc.vector.tensor_tensor(ng[:, cs], ng[:, cs], GIN[:, gsl], AD)
                    yield
                    nc.vector.tensor_scalar(ng[:, cs], ng[:, cs], -1.0, 1.0, MX, MN)
                    yield
                    # h' = ng + fg*(hq - ng)
                    nc.vector.tensor_tensor(
                        dt[:, cs], HQ[:, cs].bitcast(F32), ng[:, cs], SU
                    )
                    yield
                    nc.vector.tensor_tensor(dt[:, cs], dt[:, cs], fg[:, cs], MU)
                    yield
                    nc.vector.tensor_tensor(Hdst[:, cs], ng[:, cs], dt[:, cs], AD)
                    if t == T - 1:
                        pso = mgups.tile([NCLS, BS], F32, name="pso", tag="psg")
                        nc.tensor.matmul(
                            pso[:], FC5[:], HF[:, cs], start=True, stop=True
                        )
                        OUTS = rp.tile([NCLS, BS], F32, name="OUTS", bufs=1)
                        nc.vector.tensor_copy(OUTS[:], pso[:])
                        nc.sync.dma_start(out_d.ap(), OUTS[:])
                return emit

            # ---------------- encoder chunks
            pending = []  # deferred MGU op-generators from the previous chunk
            pending_gi = []  # deferred gi matmuls from the previous chunk

            def pump(n):
                while n > 0 and pending:
                    try:
                        next(pending[0])
                        n -= 1
                    except StopIteration:
                        pending.pop(0)

            def drain():
                while pending:
                    try:
                        next(pending[0])
                    except StopIteration:
                        pending.pop(0)
            col = 0
            t0 = 0
            for u, wpc in enumerate(wpcs):
                Sc = wpc * BS
                sl = bass.ds(col, Sc)
                last = u == len(wpcs) - 1
                A1 = a1p.tile([128, NSLOT, Sc], BF16, name="A1")
                F = fp.tile([128, NK, Sc], BF16, name="F")
                A3 = a3p.tile([128, 2, Sc], BF16, name="A3")
                psmf = [
                    f3ps.tile([128, Sc], F32, name=f"psmf{mf}", tag="ps3")
                    for mf in range(2)
                ]
                c1n = 0
                c2n = 0
                fc3_started = [False]

                def conv1_slot(s):
                    nonlocal c1n
                    ps1 = c1ps.tile([128, Sc], F32, name="ps1")
                    nc.tensor.matmul(
                        ps1[:], W1[:SP, s, :], XQ[:, sl], start=True, stop=True
                    )
                    # chunk 0 cold-start: ACT is busy with table load + DMA
                    # issue, so the first column's epilogues go to DVE
                    pat = [False] * 5 if (u == 0 and c1n < 5) else C1_PAT[wpc]
                    epilogue(A1[:, s, :], ps1[:], pat[c1n % len(pat)])
                    c1n += 1

                def conv2_mms(tiles):
                    # tiles: list of (k, [(h, s), ...]) accumulation specs,
                    # emitted h-major for lhsT reuse
                    nonlocal c2n
                    pss = {}
                    nacc = {k: len(accs) for k, accs in tiles}
                    cnt = {k: 0 for k, _ in tiles}
                    for qi in range(max(nacc.values())):
                        for k, accs in tiles:
                            if qi >= len(accs):
                                continue
                            h, s = accs[qi]
                            if k not in pss:
                                pss[k] = c2ps.tile([128, Sc], F32, name="ps2")
                            nc.tensor.matmul(
                                pss[k][:],
                                W2[:, h, :],
                                A1[:, s, :],
                                start=(cnt[k] == 0),
                                stop=(cnt[k] == nacc[k] - 1),
                                skip_group_check=True,
                            )
                            cnt[k] += 1
                    for k, _ in tiles:
                        pat = C2_PAT[wpc]
                        epilogue(F[:, k, :], pss[k][:], pat[c2n % len(pat)])
                        c2n += 1
                        for mf in range(2):
                            nc.tensor.matmul(
                                psmf[mf][:],
                                FC3[:, k, bass.ts(mf, 128)],
                                F[:, k, :],
                                start=(not fc3_started[0]),
                                stop=(k == NK - 1),
                                skip_group_check=True,
                            )
                        fc3_started[0] = True

                for cp in range(6):
                    for rw in range(5):
                        conv1_slot(cp * 5 + rw)
                    if cp == 0 and pending_gi:
                        pending_gi.pop(0)()
                    if cp >= 1:
                        jp = cp - 1  # std tiles using col pairs (jp, jp+1)
                        for ip0 in (0, 2, 4):
                            tl = [
                                (
                                    jp * 5 + ip,
                                    [(0, jp * 5 + ip), (1, cp * 5 + ip)],
                                )
                                for ip in (ip0, ip0 + 1)
                                if ip < 5
                            ]
                            conv2_mms(tl)
                    if cp == 5:
                        b = 25  # cp=5 slot base; J0=h2, J1=h3
                        conv2_mms(
                            [
                                (25, [(2, b + 0), (3, b + 1)]),
                                (26, [(2, b + 2), (3, b + 3)]),
                            ]
                        )
                        conv2_mms([(27, [(2, b + 4)])])
                    # emit deferred MGU windows as whole blocks mid-chunk
                    if cp == 1 or cp == 3:
                        pump(12)
                drain()

                # A3 = clip(fc3 psum, 0, 1)
                for mf in range(2):
                    nc.vector.tensor_scalar(
                        A3[:, mf, :], psmf[mf][:], 0.0, 1.0, MX, MN
                    )

                # gi: single M=128 accumulation pair; deferred into the next
                # chunk's PE stream so the A3-epilogue wait never stalls PE
                def gi_emit(A3=A3, sl=sl, Sc=Sc):
                    psgi = mgups.tile([128, Sc], F32, name="psgi", tag="psg")
                    for mf in range(2):
                        nc.tensor.matmul(
                            psgi[:],
                            WIH[:, mf, :],
                            A3[:, mf, :],
                            start=(mf == 0),
                            stop=(mf == 1),
                            skip_group_check=True,
                        )
                    nc.vector.tensor_scalar(
                        GIF[:, sl], psgi[0:HID, :], 0.5, 0.5, MU, AD
                    )
                    nc.vector.tensor_copy(GIN[:, sl], psgi[HID:128, :])

                if last:
                    gi_emit()
                else:
                    pending_gi.append(gi_emit)

                for w in range(wpc):
                    t = t0 + w
                    pending.append(mgu_window(t, 1)())
                if last:
                    drain()
                t0 += wpc
                col += Sc

    nc.compile()
    return nc


def _weight_map(packs):
    w1b, w2b, fc3t, wiht, whht, fc5t = packs
    return {
        "w1b": w1b, "w2b": w2b,
        "fc3t": fc3t, "wiht": wiht, "whht": whht, "fc5t": fc5t,
    }


# ---------------------------------------------------------------- entry point
def kernel(**inputs):
    x = np.asarray(inputs["x"], np.float32)
    packs = _pack_weights(
        np.asarray(inputs["conv1_w"], np.float32),
        np.asarray(inputs["conv2_w"], np.float32),
        np.asarray(inputs["fc3_w"], np.float32),
        np.asarray(inputs["w_ih"], np.float32),
        np.asarray(inputs["w_hh"], np.float32),
        np.asarray(inputs["fc5_w"], np.float32),
    )
    NCORES = 8
    B = x.shape[0]
    BS = B // NCORES

    relu_ok = _relu_safe(
        x, np.asarray(inputs["conv1_w"], np.float32),
        np.asarray(inputs["conv2_w"], np.float32),
    )
    nc = build_nc(BS=BS, relu_acts=relu_ok)
    in_maps = [dict(_weight_map(packs), xt=_pack_x(x[c * BS : (c + 1) * BS]))
               for c in range(NCORES)]
    res = run_bass_kernel_spmd(nc, in_maps, core_ids=list(range(NCORES)))
    out = np.concatenate([res.results[c]["out"].T for c in range(NCORES)], axis=0)
    return np.ascontiguousarray(out, np.float32)


if __name__ == "__main__":
    rng = np.random.default_rng(0)
    ins = {
        "x": rng.standard_normal((2048, T, HH, WW), np.float32) * 0.5,
        "conv1_w": rng.standard_normal((CH1, 1, 3, 3), np.float32) * 0.1,
        "conv2_w": rng.standard_normal((CH2, CH1, 3, 3), np.float32) * 0.1,
        "fc3_w": rng.standard_normal((256, 3520), np.float32) * 0.1,
        "w_ih": rng.standard_normal((128, 256), np.float32) * 0.1,
        "w_hh": rng.standard_normal((128, HID), np.float32) * 0.1,
        "fc5_w": rng.standard_normal((NCLS, HID), np.float32) * 0.1,
    }
    out = kernel(**ins)
    print(out.shape, out.dtype, np.abs(out).mean())
